# revision 24
# baseline (speedup 1.0000x reference)
"""Trainium2 Bass kernel for DietConv2dV2: 3x3 conv (stride 1, pad 1) + bias.

x: (16, 8, 1024, 1024) fp32, weight: (8, 8, 3, 3), bias: (8,) -> out like x.

Strategy
--------
Data-parallel: 16 images / 8 cores = 2 images per core, no collectives.

Per core the conv runs as a banded matmul on the PE array:
  - K (contraction, partitions) = 16 input rows x 8 in-channels = 128,
    partition p = hi*8 + ci.
  - M (stationary free dim)     = 14 out rows x 8 out-channels = 112,
    column  m = ho*8 + co.
  - N (moving free dim)         = 512-wide w chunk.
The stationary "band" matrix S_kw[(hi,ci),(ho,co)] = weight[co,ci,hi-ho,kw]
covers all 3 kh taps at once; the 3 kw taps are 3 PSUM-accumulated matmuls
reading the same SBUF rows at w offsets kw-1 (rows stored with 1-col zero
padding on each side).  kw is the outer loop so each stationary serves
both w-chunks back to back (3 ldweights per block, not 6 -- ldweights
does NOT hide on this walrus build, --enable-ldw-opt=false).

The 2-row block-to-block halo is re-read from HBM (2/16 of input
traffic).  Alternatives measured WORSE: an SBUF->SBUF halo copy
serializes the load pipeline through Tile's same-tile tracking + HWDGE
NX head-of-line (644us); extra K=16 boundary matmuls from the next tile
are PE-bound at ~277ns/matmul (451us).

DMA plumbing (the kernel is HBM-bound, ~358 GB/s/core cap):
  - x is declared float32r in BIR (same bits as fp32) so loads need no
    cast and can use any ring; the bulk streams on SWDGE (gpsimd), the
    first few fan out over the idle HWDGE rings to cut the startup ramp
    (SWDGE descriptor emission is ~0.7us per DMA, serial).
  - tile pad columns are pre-zeroed once; loads never write them, so
    the zeros persist across buffer reuse -- no per-block DVE memsets
    serializing ahead of the loads (only the 2 image-edge blocks).
  - output writes alternate both HWDGE rings, fanning out over
    sync/scalar/gpsimd for the last blocks (write-only tail).  A
    permanent 3-ring write rotation measured neutral (the SWDGE queue
    becomes the long pole), and write PAIRING via a 4-dim
    (two ho) c w AP is rejected by the DMA balancer (3 dims max).
"""

import numpy as np

import bass_rust
import concourse.bass as bass
import concourse.mybir as mybir
from concourse.tile import TileContext
from concourse.bass_utils import run_bass_kernel_spmd

F32 = mybir.dt.float32
F32R = mybir.dt.float32r

N_CORES = 8
IMG_PER_CORE = 2
C = 8          # channels (in == out)
H = 1024
W = 1024
KS = 3         # kernel size
HB = 14        # output rows per block (16 input rows -> 14 output rows)
KROWS = HB + KS - 1  # 16 input rows per block
M = C * HB     # 112 stationary columns
WCHUNK = 512   # PSUM bank = 512 fp32
XBUFS = 16


def _split_excess_waits(nc):
    """This walrus build accepts 1 sync-wait per instruction (2 for
    EventSemaphore); Tile's final drain and ldweights can end up with
    more.  Move overflow waits onto EventSemaphore carriers inserted
    before the offender on the same engine."""
    for fn in nc.m.functions:
        for blk in fn.blocks:
            out = []
            changed = False
            for inst in blk.instructions:
                si = inst.sync_info
                cap = 2 if inst.opcode == "EventSemaphore" else 1
                waits = list(si.on_wait) if si is not None else []
                if len(waits) > cap:
                    changed = True
                    overflow, keep = waits[:-cap], waits[-cap:]
                    for j in range(0, len(overflow), 2):
                        es = mybir.InstEventSemaphore(
                            name=nc.get_next_instruction_name(), ins=[], outs=[]
                        )
                        es.engine = inst.engine
                        es.sync_info = bass_rust.SyncInfo(
                            on_wait=overflow[j : j + 2], on_update=[]
                        )
                        nc.register_instruction(es, overwrite=True)
                        out.append(es)
                    inst.sync_info = bass_rust.SyncInfo(
                        on_wait=keep, on_update=list(si.on_update)
                    )
                out.append(inst)
            if changed:
                blk.instructions = out


def _build(nimg, h, w, reps=1, salt=0):
    nblocks = -(-h // HB)        # 74
    nchunks = w // WCHUNK        # 2
    nall = nimg * nblocks

    nc = bass.Bass(name=f"dietconv_s{salt}")
    # x is declared float32r: identical bits (np maps f32r -> float32),
    # but dtype-matching the SBUF tiles lets loads ride the HWDGE rings
    # (a fp32->fp32r "cast" DMA is SWDGE-only).  The PE's fp32r rounding
    # happens in its datapath either way (verified: rel err unchanged).
    x = nc.dram_tensor("x", [nimg, C, h, w], F32R, kind="ExternalInput")
    wb = nc.dram_tensor("wband", [KS, 128, M], F32R, kind="ExternalInput")
    bv = nc.dram_tensor("biasv", [M, 1], F32, kind="ExternalInput")
    out = nc.dram_tensor("out", [nimg, C, h, w], F32, kind="ExternalOutput")

    # row-major (h, c) views so SBUF partition p = hi*8 + ci
    xr = x.rearrange("n c h w -> n h c w")
    outr = out.rearrange("n c h w -> n h c w")

    with TileContext(nc) as tc:
        with (
            tc.tile_pool(name="wpool", bufs=1) as wpool,
            tc.tile_pool(name="xpool", bufs=XBUFS) as xpool,
            tc.tile_pool(name="opool", bufs=8) as opool,
            tc.tile_pool(name="pspool", bufs=4, space="PSUM") as pspool,
        ):
            wts = []
            for kw in range(KS):
                wt = wpool.tile([128, M], F32R, name=f"wt{kw}")
                nc.scalar.dma_start(out=wt[:], in_=wb[kw])
                wts.append(wt)
            bt = wpool.tile([M, 1], F32, name="bt")
            nc.sync.dma_start(out=bt[:], in_=bv[:])

            def body():
                # pre-zero the pad columns (0 and w+1) of every x buffer
                # once; loads only ever write cols 1..w+1, so the zeros
                # persist across all reuses of the rotating buffers.
                for _ in range(XBUFS):
                    xt = xpool.tile([128, w + 2], F32R, name="xt")
                    nc.vector.memset(xt[:, 0:1].bitcast(F32), 0.0)
                    nc.vector.memset(xt[:, w + 1 : w + 2].bitcast(F32), 0.0)

                tiles = {}

                def load(g, eng=None):
                    n, t = divmod(g, nblocks)
                    h0 = t * HB
                    hlo = h0 - 1  # input rows [hlo, hlo + 16)
                    vlo = max(hlo, 0)
                    vhi = min(hlo + KROWS, h)
                    plo = (vlo - hlo) * C
                    phi = (vhi - hlo) * C
                    xt = xpool.tile([128, w + 2], F32R, name="xt")
                    # out-of-image rows exist only in the first/last
                    # block of each image; zero them (DVE partition
                    # start must be 32-aligned -- widen, the DMA below
                    # rewrites the valid rows; fp32r isn't a DVE dtype,
                    # memset via fp32 bitcast).
                    if plo > 0:
                        nc.vector.memset(xt[0:plo, :].bitcast(F32), 0.0)
                    if phi < 128:
                        alo = (phi // 32) * 32
                        nc.vector.memset(xt[alo:128, :].bitcast(F32), 0.0)
                    (eng or nc.gpsimd).dma_start(
                        out=xt[plo:phi, 1 : w + 1], in_=xr[n, vlo:vhi, :, :]
                    )
                    tiles[g] = xt

                # ramp: SWDGE emission costs ~0.7us per DMA, so the
                # first loads fan out over the (idle) HWDGE rings too
                ramp_engines = [nc.sync, nc.scalar, None, nc.sync, nc.scalar, None]
                for g0 in range(6):
                    load(g0, eng=ramp_engines[g0])
                for g in range(nall):
                    n, t = divmod(g, nblocks)
                    if g + 6 < nall:
                        load(g + 6)
                    h0 = t * HB
                    nho = min(HB, h - h0)
                    xt = tiles.pop(g)
                    # one PSUM tile spanning both w-chunks (2 banks);
                    # each matmul stays within one bank
                    ps = pspool.tile([M, w], F32, name="ps", tag="ps")
                    for kw in range(KS):
                        for j in range(nchunks):
                            c0 = j * WCHUNK + kw
                            nc.tensor.matmul(
                                ps[:, j * WCHUNK : (j + 1) * WCHUNK],
                                wts[kw][:],
                                xt[:, c0 : c0 + WCHUNK],
                                start=(kw == 0),
                                stop=(kw == KS - 1),
                            )
                    ot = opool.tile([M, w], F32, name="ot", tag="ot")
                    nc.vector.tensor_scalar_add(ot[:], ps[:], bt[:])
                    # alternate output DMAs across both HWDGE rings
                    # (~70us faster than one ring), route every 6th
                    # write through SWDGE: the 2-ring write stream runs
                    # ~5 GB/s under the pace that would finish it with
                    # the reads, so ~11MB of writes shift to gpsimd to
                    # converge the two streams (1/3 on gpsimd measured
                    # neutral-worse: SWDGE becomes the long pole; the
                    # 6-block load emission lead covers the Q7 stall on
                    # the bias-add semaphore).  Fan out over all three
                    # rings at the tail, when the input stream is done.
                    if g >= nall - 6 or g % 6 == 5:
                        dma_eng = (nc.sync, nc.scalar, nc.gpsimd)[g % 3]
                    else:
                        dma_eng = nc.scalar if g % 2 == 0 else nc.sync
                    dma_eng.dma_start(
                        out=outr[n, h0 : h0 + nho, :, :],
                        in_=ot[0 : nho * C, :],
                    )

            # static unroll: tc.For_i loop control hits a walrus codegen
            # gap in this build ("ISA wrong length" on CompareAndBranch)
            for _ in range(reps):
                body()

    _split_excess_waits(nc)
    return nc


def _band_inputs(weight, bias):
    weight = np.asarray(weight, dtype=np.float32)
    bias = np.asarray(bias, dtype=np.float32)
    S = np.zeros((KS, 128, M), dtype=np.float32)
    for kw in range(KS):
        for kh in range(KS):
            blk = weight[:, :, kh, kw].T  # [ci, co]
            for ho in range(HB):
                hi = ho + kh
                S[kw, hi * C : (hi + 1) * C, ho * C : (ho + 1) * C] = blk
    biasv = np.tile(bias, HB).astype(np.float32)[:, None]  # m = ho*8 + co
    return S, biasv


def _in_maps(x, weight, bias, nimg_per_core, n_cores):
    S, biasv = _band_inputs(weight, bias)
    x = np.ascontiguousarray(x, dtype=np.float32)
    return [
        {
            "x": x[i * nimg_per_core : (i + 1) * nimg_per_core],
            "wband": S,
            "biasv": biasv,
        }
        for i in range(n_cores)
    ]


def _run(x, weight, bias, nimg_per_core, h, w, n_cores, reps=1):
    in_maps = _in_maps(x, weight, bias, nimg_per_core, n_cores)
    # The walrus backend compile is rarely flaky (parallel codegen race).
    # jax caches the failed compilation by HLO, so retries must change the
    # BIR bytes (salt) and drop the jit cache.
    last_exc = None
    for attempt in range(4):
        try:
            nc = _build(nimg_per_core, h, w, reps, salt=attempt)
            res = run_bass_kernel_spmd(nc, in_maps, core_ids=list(range(n_cores)))
            break
        except Exception as e:  # noqa: BLE001
            last_exc = e
            try:
                import jax

                jax.clear_caches()
            except Exception:  # noqa: BLE001
                pass
    else:
        raise last_exc
    return np.concatenate([r["out"] for r in res.results], axis=0)


def kernel(x, weight, bias):
    return _run(x, weight, bias, IMG_PER_CORE, H, W, N_CORES, reps=1)


# revision 27
# speedup vs baseline: 1.1304x; 1.1304x over previous
"""Trainium2 Bass kernel for DietConv2dV2: 3x3 conv (stride 1, pad 1) + bias.

x: (16, 8, 1024, 1024) fp32, weight: (8, 8, 3, 3), bias: (8,) -> out like x.

Strategy
--------
Data-parallel: 16 images / 8 cores = 2 images per core, no collectives.

Per core the conv runs as a banded matmul on the PE array:
  - K (contraction, partitions) = 16 input rows x 8 in-channels = 128,
    partition p = hi*8 + ci.
  - M (stationary free dim)     = 14 out rows x 8 out-channels = 112,
    column  m = ho*8 + co.
  - N (moving free dim)         = 512-wide w chunk.
The stationary "band" matrix S_kw[(hi,ci),(ho,co)] = weight[co,ci,hi-ho,kw]
covers all 3 kh taps at once; the 3 kw taps are 3 PSUM-accumulated matmuls
reading the same SBUF rows at w offsets kw-1 (rows stored with 1-col zero
padding on each side).  kw is the outer loop so each stationary serves
both w-chunks back to back (3 ldweights per block, not 6 -- ldweights
does NOT hide on this walrus build, --enable-ldw-opt=false).

The 2-row block-to-block halo is re-read from HBM (2/16 of input
traffic).  Alternatives measured WORSE: an SBUF->SBUF halo copy
serializes the load pipeline through Tile's same-tile tracking + HWDGE
NX head-of-line (644us); extra K=16 boundary matmuls from the next tile
are PE-bound at ~277ns/matmul (451us).

DMA plumbing (the kernel is HBM-bound, ~358 GB/s/core cap):
  - x is declared float32r in BIR (same bits as fp32) so loads need no
    cast and can use any ring; the bulk streams on SWDGE (gpsimd), the
    first few fan out over the idle HWDGE rings to cut the startup ramp
    (SWDGE descriptor emission is ~0.7us per DMA, serial).
  - tile pad columns are pre-zeroed once; loads never write them, so
    the zeros persist across buffer reuse -- no per-block DVE memsets
    serializing ahead of the loads (only the 2 image-edge blocks).
  - output writes alternate both HWDGE rings, fanning out over
    sync/scalar/gpsimd for the last blocks (write-only tail).  A
    permanent 3-ring write rotation measured neutral (the SWDGE queue
    becomes the long pole), and write PAIRING via a 4-dim
    (two ho) c w AP is rejected by the DMA balancer (3 dims max).
"""

import numpy as np

import bass_rust
import concourse.bass as bass
import concourse.mybir as mybir
from concourse.tile import TileContext
from concourse.bass_utils import run_bass_kernel_spmd

F32 = mybir.dt.float32
F32R = mybir.dt.float32r

N_CORES = 8
IMG_PER_CORE = 2
C = 8          # channels (in == out)
H = 1024
W = 1024
KS = 3         # kernel size
HB = 14        # output rows per block (16 input rows -> 14 output rows)
KROWS = HB + KS - 1  # 16 input rows per block
M = C * HB     # 112 stationary columns
WCHUNK = 512   # PSUM bank = 512 fp32
XBUFS = 16


def _split_excess_waits(nc):
    """This walrus build accepts 1 sync-wait per instruction (2 for
    EventSemaphore); Tile's final drain and ldweights can end up with
    more.  Move overflow waits onto EventSemaphore carriers inserted
    before the offender on the same engine."""
    for fn in nc.m.functions:
        for blk in fn.blocks:
            out = []
            changed = False
            for inst in blk.instructions:
                si = inst.sync_info
                cap = 2 if inst.opcode == "EventSemaphore" else 1
                waits = list(si.on_wait) if si is not None else []
                if len(waits) > cap:
                    changed = True
                    overflow, keep = waits[:-cap], waits[-cap:]
                    for j in range(0, len(overflow), 2):
                        es = mybir.InstEventSemaphore(
                            name=nc.get_next_instruction_name(), ins=[], outs=[]
                        )
                        es.engine = inst.engine
                        es.sync_info = bass_rust.SyncInfo(
                            on_wait=overflow[j : j + 2], on_update=[]
                        )
                        nc.register_instruction(es, overwrite=True)
                        out.append(es)
                    inst.sync_info = bass_rust.SyncInfo(
                        on_wait=keep, on_update=list(si.on_update)
                    )
                out.append(inst)
            if changed:
                blk.instructions = out


def _build(nimg, h, w, reps=1, salt=0):
    nblocks = -(-h // HB)        # 74
    nchunks = w // WCHUNK        # 2
    nall = nimg * nblocks

    nc = bass.Bass(name=f"dietconv_s{salt}")
    # x is declared float32r: identical bits (np maps f32r -> float32),
    # but dtype-matching the SBUF tiles lets loads ride the HWDGE rings
    # (a fp32->fp32r "cast" DMA is SWDGE-only).  The PE's fp32r rounding
    # happens in its datapath either way (verified: rel err unchanged).
    x = nc.dram_tensor("x", [nimg, C, h, w], F32R, kind="ExternalInput")
    wb = nc.dram_tensor("wband", [KS, 128, M], F32R, kind="ExternalInput")
    bv = nc.dram_tensor("biasv", [M, 1], F32, kind="ExternalInput")
    out = nc.dram_tensor("out", [nimg, C, h, w], F32, kind="ExternalOutput")

    # row-major (h, c) views so SBUF partition p = hi*8 + ci
    xr = x.rearrange("n c h w -> n h c w")
    outr = out.rearrange("n c h w -> n h c w")

    with TileContext(nc) as tc:
        with (
            tc.tile_pool(name="wpool", bufs=1) as wpool,
            tc.tile_pool(name="xpool", bufs=XBUFS) as xpool,
            tc.tile_pool(name="opool", bufs=8) as opool,
            tc.tile_pool(name="pspool", bufs=4, space="PSUM") as pspool,
        ):
            wts = []
            for kw in range(KS):
                wt = wpool.tile([128, M], F32R, name=f"wt{kw}")
                nc.scalar.dma_start(out=wt[:], in_=wb[kw])
                wts.append(wt)
            bt = wpool.tile([M, 1], F32, name="bt")
            nc.sync.dma_start(out=bt[:], in_=bv[:])

            def body():
                # pre-zero the pad columns (0 and w+1) of every x buffer
                # once; loads only ever write cols 1..w+1, so the zeros
                # persist across all reuses of the rotating buffers.
                for _ in range(XBUFS):
                    xt = xpool.tile([128, w + 2], F32R, name="xt")
                    nc.vector.memset(xt[:, 0:1].bitcast(F32), 0.0)
                    nc.vector.memset(xt[:, w + 1 : w + 2].bitcast(F32), 0.0)

                tiles = {}

                def load(g, eng=None):
                    n, t = divmod(g, nblocks)
                    h0 = t * HB
                    hlo = h0 - 1  # input rows [hlo, hlo + 16)
                    vlo = max(hlo, 0)
                    vhi = min(hlo + KROWS, h)
                    plo = (vlo - hlo) * C
                    phi = (vhi - hlo) * C
                    xt = xpool.tile([128, w + 2], F32R, name="xt")
                    # out-of-image rows exist only in the first/last
                    # block of each image; zero them (DVE partition
                    # start must be 32-aligned -- widen, the DMA below
                    # rewrites the valid rows; fp32r isn't a DVE dtype,
                    # memset via fp32 bitcast).
                    if plo > 0:
                        nc.vector.memset(xt[0:plo, :].bitcast(F32), 0.0)
                    if phi < 128:
                        alo = (phi // 32) * 32
                        nc.vector.memset(xt[alo:128, :].bitcast(F32), 0.0)
                    (eng or nc.gpsimd).dma_start(
                        out=xt[plo:phi, 1 : w + 1], in_=xr[n, vlo:vhi, :, :]
                    )
                    tiles[g] = xt

                # ramp: SWDGE emission costs ~0.7us per DMA, so the
                # first loads fan out over the (idle) HWDGE rings too
                ramp_engines = [nc.sync, nc.scalar, None, nc.sync, nc.scalar, None]
                for g0 in range(6):
                    load(g0, eng=ramp_engines[g0])
                for g in range(nall):
                    n, t = divmod(g, nblocks)
                    if g + 6 < nall:
                        if g < nall - 16:
                            load(g + 6)
                        elif g == nall - 16:
                            # emit every remaining load now: their WAR
                            # semaphores (readers of buffer g-16) are
                            # already clear, so Q7 fires them straight
                            # away, and in program order they all sit
                            # AHEAD of the tail's gpsimd writes -- the
                            # 3-ring fanout below can't stall the load
                            # stream (the v9 failure mode).
                            for gg in range(g + 6, nall):
                                load(gg)
                    h0 = t * HB
                    nho = min(HB, h - h0)
                    xt = tiles.pop(g)
                    # one PSUM tile spanning both w-chunks (2 banks);
                    # each matmul stays within one bank
                    ps = pspool.tile([M, w], F32, name="ps", tag="ps")
                    for kw in range(KS):
                        for j in range(nchunks):
                            c0 = j * WCHUNK + kw
                            nc.tensor.matmul(
                                ps[:, j * WCHUNK : (j + 1) * WCHUNK],
                                wts[kw][:],
                                xt[:, c0 : c0 + WCHUNK],
                                start=(kw == 0),
                                stop=(kw == KS - 1),
                            )
                    ot = opool.tile([M, w], F32, name="ot", tag="ot")
                    nc.vector.tensor_scalar_add(ot[:], ps[:], bt[:])
                    # alternate output DMAs across both HWDGE rings
                    # (~70us faster than one ring); fan out over all
                    # three at the tail, when the input stream is done
                    # (a permanent 3-ring rotation measured neutral:
                    # the SWDGE queue just becomes the long pole).
                    if g >= nall - 12:
                        dma_eng = (nc.sync, nc.scalar, nc.gpsimd)[g % 3]
                    else:
                        dma_eng = nc.scalar if g % 2 == 0 else nc.sync
                    dma_eng.dma_start(
                        out=outr[n, h0 : h0 + nho, :, :],
                        in_=ot[0 : nho * C, :],
                    )

            # static unroll: tc.For_i loop control hits a walrus codegen
            # gap in this build ("ISA wrong length" on CompareAndBranch)
            for _ in range(reps):
                body()

    _split_excess_waits(nc)
    return nc


def _band_inputs(weight, bias):
    weight = np.asarray(weight, dtype=np.float32)
    bias = np.asarray(bias, dtype=np.float32)
    S = np.zeros((KS, 128, M), dtype=np.float32)
    for kw in range(KS):
        for kh in range(KS):
            blk = weight[:, :, kh, kw].T  # [ci, co]
            for ho in range(HB):
                hi = ho + kh
                S[kw, hi * C : (hi + 1) * C, ho * C : (ho + 1) * C] = blk
    biasv = np.tile(bias, HB).astype(np.float32)[:, None]  # m = ho*8 + co
    return S, biasv


def _in_maps(x, weight, bias, nimg_per_core, n_cores):
    S, biasv = _band_inputs(weight, bias)
    x = np.ascontiguousarray(x, dtype=np.float32)
    return [
        {
            "x": x[i * nimg_per_core : (i + 1) * nimg_per_core],
            "wband": S,
            "biasv": biasv,
        }
        for i in range(n_cores)
    ]


def _run(x, weight, bias, nimg_per_core, h, w, n_cores, reps=1):
    in_maps = _in_maps(x, weight, bias, nimg_per_core, n_cores)
    # The walrus backend compile is rarely flaky (parallel codegen race).
    # jax caches the failed compilation by HLO, so retries must change the
    # BIR bytes (salt) and drop the jit cache.
    last_exc = None
    for attempt in range(4):
        try:
            nc = _build(nimg_per_core, h, w, reps, salt=attempt)
            res = run_bass_kernel_spmd(nc, in_maps, core_ids=list(range(n_cores)))
            break
        except Exception as e:  # noqa: BLE001
            last_exc = e
            try:
                import jax

                jax.clear_caches()
            except Exception:  # noqa: BLE001
                pass
    else:
        raise last_exc
    return np.concatenate([r["out"] for r in res.results], axis=0)


def kernel(x, weight, bias):
    return _run(x, weight, bias, IMG_PER_CORE, H, W, N_CORES, reps=1)
